# revision 4
# baseline (speedup 1.0000x reference)
"""Multi-head attention (GAttention) on 8 trn2 NeuronCores.

Reference computation (per batch b):
    q = x @ w_qkv.T            -> [N, 768], heads of 64
    attn = softmax(q k^T / 8)  -> per head [N, M]
    out_h = attn @ v           -> [N, 64]
    out = concat(out_h) @ w_proj.T + b_proj

Sharding: 24 (b, head) units over 8 cores -> each core gets one batch b and
3 heads. Each core computes its heads' attention plus its partial
projection sum [N, 768]; host adds the 4 partials per batch + bias.

Per-core device pipeline (all matmuls f32r: 1 cyc/row, ~2.4e-4 rel rounding):
  1. qproj: qT[h] = wq_h^T-slices^T @ x^T        (contraction over C=768)
  2. per head, per key-tile mi (128 keys), per n-half (1024 queries):
       S^T[m, n] = k_h q^T (contraction d=64)    -> PSUM [128, 1024]
       expT = exp(0.125 * S^T)                   -> SBUF f32r (ACT, fused scale)
       AV: outT[128, n] += v_aug[m,128]^T expT   -> PSUM accumulate over mi
     where v_aug = [v_h | ones*64] so PSUM rows 64:128 hold the softmax
     denominator replicated; rows 0:64 hold unnormalized out^T.
  3. normalize: outTn = outT[0:64] * recip(outT[64:128])  (DVE)
  4. proj: out[n-tile, 768] = sum_h outTn_h^T @ wp_h      (PSUM accum over h)
"""
import numpy as np
from contextlib import ExitStack

import concourse.bass as bass
import concourse.mybir as mybir
import concourse.tile as tile
from concourse import bacc
from concourse.bass_utils import run_bass_kernel_spmd

B, N, DIM = 2, 2048, 768
H, D = 12, 64
M = 2048
NCORES = 8
HPC = 3            # heads per core
NT = N // 128      # 16 query tiles
MT = M // 128      # 16 key tiles
CT = DIM // 128    # 6 contraction tiles for qproj
NHALF = 1024       # exp/psum n-granularity
F32 = mybir.dt.float32
F32R = mybir.dt.float32r

_cached = {}


def build_program():
    nc = bacc.Bacc("TRN2", target_bir_lowering=False, debug=False)
    xT_d = nc.dram_tensor("xT", [DIM, N], F32R, kind="ExternalInput")
    wq_d = nc.dram_tensor("wq", [HPC, DIM, D], F32R, kind="ExternalInput")
    kT_d = nc.dram_tensor("kT", [HPC, D, M], F32R, kind="ExternalInput")
    va_d = nc.dram_tensor("va", [HPC, M, 128], F32R, kind="ExternalInput")
    wp_d = nc.dram_tensor("wp", [HPC, D, DIM], F32R, kind="ExternalInput")
    out_d = nc.dram_tensor("out", [N, DIM], F32, kind="ExternalOutput")

    with tile.TileContext(nc) as tc, ExitStack() as ctx:
        big = ctx.enter_context(tc.tile_pool(name="big", bufs=1))
        expp = ctx.enter_context(tc.tile_pool(name="expp", bufs=3))
        nrm = ctx.enter_context(tc.tile_pool(name="nrm", bufs=2))
        stg = ctx.enter_context(tc.tile_pool(name="stg", bufs=3))

        # persistent SBUF tensors
        xT_t = [big.tile([128, N], F32R, name=f"xT{c}", tag=f"xT{c}")
                for c in range(CT)]
        for c in range(CT):
            nc.sync.dma_start(xT_t[c][:], xT_d[c * 128:(c + 1) * 128, :])
        wq_t = big.tile([128, HPC, CT, D], F32R)
        nc.sync.dma_start(
            wq_t[:], wq_d.rearrange("h (c p) d -> p h c d", p=128))
        kT_t = big.tile([64, HPC, M], F32R)
        nc.sync.dma_start(kT_t[:], kT_d.rearrange("h d m -> d h m"))
        va_t = big.tile([128, HPC, MT, 128], F32R)
        nc.sync.dma_start(
            va_t[:], va_d.rearrange("h (t p) e -> p h t e", p=128))
        wp_t = big.tile([64, HPC, DIM], F32R)
        nc.sync.dma_start(wp_t[:], wp_d.rearrange("h d o -> d h o"))
        qT_t = big.tile([64, HPC, N], F32R)
        outTn_t = big.tile([64, HPC, N], F32R)

        # phase 1: q projection, per head: qT[64, N] with PSUM accum over C
        with tc.tile_pool(name="qp_ps", bufs=1, space="PSUM") as qp_ps:
            for h in range(HPC):
                qp = qp_ps.tile([64, N], F32)
                for c in range(CT):
                    for ch in range(N // 512):
                        nc.tensor.matmul(
                            qp[:, ch * 512:(ch + 1) * 512],
                            wq_t[:, h, c, :],
                            xT_t[c][:, ch * 512:(ch + 1) * 512],
                            start=(c == 0), stop=(c == CT - 1),
                        )
                nc.vector.tensor_copy(qT_t[:, h, :], qp[:])

        # phase 2: attention per head
        with tc.tile_pool(name="st_ps", bufs=2, space="PSUM") as st_ps, \
             tc.tile_pool(name="av_ps", bufs=1, space="PSUM") as av_ps:
            for h in range(HPC):
                av = av_ps.tile([128, N], F32, tag="av")
                for mi in range(MT):
                    kslice = kT_t[:, h, mi * 128:(mi + 1) * 128]
                    for half in range(N // NHALF):
                        st = st_ps.tile([128, NHALF], F32, tag="st")
                        for cc in range(NHALF // 512):
                            n0 = half * NHALF + cc * 512
                            nc.tensor.matmul(
                                st[:, cc * 512:(cc + 1) * 512],
                                kslice,
                                qT_t[:, h, n0:n0 + 512],
                                start=True, stop=True,
                            )
                        et = expp.tile([128, NHALF], F32R, tag="et")
                        nc.scalar.activation(
                            et[:], st[:],
                            mybir.ActivationFunctionType.Exp,
                            scale=float(D) ** -0.5,
                        )
                        for cc in range(NHALF // 512):
                            n0 = half * NHALF + cc * 512
                            nc.tensor.matmul(
                                av[:, n0:n0 + 512],
                                va_t[:, h, mi, :],
                                et[:, cc * 512:(cc + 1) * 512],
                                start=(mi == 0), stop=(mi == MT - 1),
                            )
                # normalize: rows 64:128 of av hold the denominator
                rs = nrm.tile([64, N], F32, tag="rs")
                nc.vector.reciprocal(rs[:], av[64:128, :])
                nc.vector.tensor_mul(outTn_t[:, h, :], av[0:64, :], rs[:])

        # phase 3: projection, PSUM accumulates the 3 heads
        with tc.tile_pool(name="pj_ps", bufs=2, space="PSUM") as pj_ps:
            for ni in range(NT):
                pp = pj_ps.tile([128, 2, 512], F32, tag="pp")
                for h in range(HPC):
                    for oc in range(2):
                        nc.tensor.matmul(
                            pp[:, oc, 0:384],
                            outTn_t[:, h, ni * 128:(ni + 1) * 128],
                            wp_t[:, h, oc * 384:(oc + 1) * 384],
                            start=(h == 0), stop=(h == HPC - 1),
                        )
                os_t = stg.tile([128, DIM], F32, tag="os")
                nc.scalar.copy(os_t[:, 0:384], pp[:, 0, 0:384])
                nc.scalar.copy(os_t[:, 384:768], pp[:, 1, 0:384])
                nc.sync.dma_start(out_d[ni * 128:(ni + 1) * 128, :], os_t[:])

    nc.compile()
    return nc


def build_in_maps(x, k, v, w_qkv, w_proj):
    x = np.asarray(x, dtype=np.float32)
    k = np.asarray(k, dtype=np.float32)
    v = np.asarray(v, dtype=np.float32)
    wqT = np.ascontiguousarray(np.asarray(w_qkv, np.float32).T)   # [C, 768]
    wpT = np.ascontiguousarray(np.asarray(w_proj, np.float32).T)  # [768, 768]

    in_maps = []
    for core in range(NCORES):
        b = core // 4
        hs = [3 * (core % 4) + i for i in range(HPC)]
        xT = np.ascontiguousarray(x[b].T)                        # [DIM, N]
        wq = np.stack([wqT[:, 64 * h:64 * (h + 1)] for h in hs])  # [3, DIM, D]
        kT = np.ascontiguousarray(k[b, hs].transpose(0, 2, 1))   # [3, D, M]
        va = np.ones((HPC, M, 128), dtype=np.float32)
        va[:, :, :D] = v[b, hs]                                  # [3, M, 128]
        wp = np.stack([wpT[64 * h:64 * (h + 1), :] for h in hs])  # [3, D, DIM]
        in_maps.append({"xT": xT, "wq": wq, "kT": kT,
                        "va": np.ascontiguousarray(va),
                        "wp": np.ascontiguousarray(wp)})
    return in_maps


def kernel(x, k, v, w_qkv, w_proj, b_proj):
    b_proj = np.asarray(b_proj, dtype=np.float32)

    if "nc" not in _cached:
        _cached["nc"] = build_program()
    nc = _cached["nc"]

    in_maps = build_in_maps(x, k, v, w_qkv, w_proj)
    res = run_bass_kernel_spmd(nc, in_maps, core_ids=list(range(NCORES)))

    out = np.empty((B, N, DIM), dtype=np.float32)
    for b in range(B):
        acc = np.zeros((N, DIM), dtype=np.float64)
        for core in range(4 * b, 4 * b + 4):
            acc += res.results[core]["out"]
        out[b] = (acc + b_proj).astype(np.float32)
    return out


# revision 8
# speedup vs baseline: 1.0279x; 1.0279x over previous
"""Multi-head attention (GAttention) on 8 trn2 NeuronCores.

Reference computation (per batch b):
    q = x @ w_qkv.T            -> [N, 768], heads of 64
    attn = softmax(q k^T / 8)  -> per head [N, M]
    out_h = attn @ v           -> [N, 64]
    out = concat(out_h) @ w_proj.T + b_proj

Sharding: 24 (b, head) units over 8 cores -> each core gets one batch b and
3 heads. Each core computes its heads' attention plus its partial
projection sum [N, 768]; host adds the 4 partials per batch + bias.

Per-core device pipeline (all matmuls f32r: 1 cyc/row, ~2.4e-4 rel rounding):
  1. qproj: qT[h] = wq_h^T-slices^T @ x^T        (contraction over C=768)
  2. per head, per key-tile mi (128 keys), per n-half (1024 queries):
       S^T[m, n] = k_h q^T (contraction d=64)    -> PSUM [128, 1024]
       expT = exp(0.125 * S^T)                   -> SBUF f32r (ACT, fused scale)
       AV: outT[128, n] += v_aug[m,128]^T expT   -> PSUM accumulate over mi
     where v_aug = [v_h | ones*64] so PSUM rows 64:128 hold the softmax
     denominator replicated; rows 0:64 hold unnormalized out^T.
  3. normalize: outTn = outT[0:64] * recip(outT[64:128])  (DVE)
  4. proj: out[n-tile, 768] = sum_h outTn_h^T @ wp_h      (PSUM accum over h)
"""
import numpy as np
from contextlib import ExitStack

import concourse.bass as bass
import concourse.mybir as mybir
import concourse.tile as tile
from concourse import bacc
from concourse.bass_utils import run_bass_kernel_spmd

B, N, DIM = 2, 2048, 768
H, D = 12, 64
M = 2048
NCORES = 8
HPC = 3            # heads per core
NT = N // 128      # 16 query tiles
MT = M // 128      # 16 key tiles
CT = DIM // 128    # 6 contraction tiles for qproj
NHALF = 1024       # exp/psum n-granularity
F32 = mybir.dt.float32
F32R = mybir.dt.float32r

_cached = {}


def build_program():
    nc = bacc.Bacc("TRN2", target_bir_lowering=False, debug=False)
    xT_d = nc.dram_tensor("xT", [DIM, N], F32R, kind="ExternalInput")
    wq_d = nc.dram_tensor("wq", [HPC, DIM, D], F32R, kind="ExternalInput")
    kT_d = nc.dram_tensor("kT", [HPC, D, M], F32R, kind="ExternalInput")
    va_d = nc.dram_tensor("va", [HPC, M, 128], F32R, kind="ExternalInput")
    wp_d = nc.dram_tensor("wp", [HPC, D, DIM], F32R, kind="ExternalInput")
    out_d = nc.dram_tensor("out", [N, DIM], F32, kind="ExternalOutput")

    with tile.TileContext(nc) as tc, ExitStack() as ctx:
        big = ctx.enter_context(tc.tile_pool(name="big", bufs=1))
        expp = ctx.enter_context(tc.tile_pool(name="expp", bufs=3))
        stg = ctx.enter_context(tc.tile_pool(name="stg", bufs=3))

        # persistent SBUF tensors
        xT_t = [big.tile([128, N], F32R, name=f"xT{c}", tag=f"xT{c}")
                for c in range(CT)]
        for c in range(CT):
            nc.sync.dma_start(xT_t[c][:], xT_d[c * 128:(c + 1) * 128, :])
        wq_t = big.tile([128, HPC, CT, D], F32R)
        nc.sync.dma_start(
            wq_t[:], wq_d.rearrange("h (c p) d -> p h c d", p=128))
        kT_t = big.tile([64, HPC, M], F32R)
        nc.sync.dma_start(kT_t[:], kT_d.rearrange("h d m -> d h m"))
        va_t = big.tile([128, HPC, MT, 128], F32R)
        nc.sync.dma_start(
            va_t[:], va_d.rearrange("h (t p) e -> p h t e", p=128))
        wp_t = big.tile([64, HPC, DIM], F32R)
        nc.sync.dma_start(wp_t[:], wp_d.rearrange("h d o -> d h o"))
        qT_t = big.tile([64, HPC, N], F32R)
        outTn_t = big.tile([64, HPC, N], F32R)

        # phase 1: q projection, per head: qT[64, N] with PSUM accum over C
        with tc.tile_pool(name="qp_ps", bufs=1, space="PSUM") as qp_ps:
            for h in range(HPC):
                qp = qp_ps.tile([64, N], F32)
                for c in range(CT):
                    for ch in range(N // 512):
                        nc.tensor.matmul(
                            qp[:, ch * 512:(ch + 1) * 512],
                            wq_t[:, h, c, :],
                            xT_t[c][:, ch * 512:(ch + 1) * 512],
                            start=(c == 0), stop=(c == CT - 1),
                        )
                nc.vector.tensor_copy(qT_t[:, h, :], qp[:])

        # phase 2: attention in 6 (head, n-half) units, pipelined via bufs=2
        with tc.tile_pool(name="st_ps", bufs=2, space="PSUM") as st_ps, \
             tc.tile_pool(name="av_ps", bufs=2, space="PSUM") as av_ps:
            for h in range(HPC):
                for half in range(N // NHALF):
                    av = av_ps.tile([128, NHALF], F32, tag="av")
                    for mi in range(MT):
                        kslice = kT_t[:, h, mi * 128:(mi + 1) * 128]
                        st = st_ps.tile([128, NHALF], F32, tag="st")
                        for cc in range(NHALF // 512):
                            n0 = half * NHALF + cc * 512
                            nc.tensor.matmul(
                                st[:, cc * 512:(cc + 1) * 512],
                                kslice,
                                qT_t[:, h, n0:n0 + 512],
                                start=True, stop=True,
                            )
                        et = expp.tile([128, NHALF], F32R, tag="et")
                        nc.scalar.activation(
                            et[:], st[:],
                            mybir.ActivationFunctionType.Exp,
                            scale=float(D) ** -0.5,
                        )
                        for cc in range(NHALF // 512):
                            nc.tensor.matmul(
                                av[:, cc * 512:(cc + 1) * 512],
                                va_t[:, h, mi, :],
                                et[:, cc * 512:(cc + 1) * 512],
                                start=(mi == 0), stop=(mi == MT - 1),
                            )
                    # rows 64:128 of av hold the softmax denominator
                    rs = expp.tile([64, NHALF], F32, tag="rs")
                    nc.vector.reciprocal(rs[:], av[64:128, :])
                    nc.vector.tensor_mul(
                        outTn_t[:, h, half * NHALF:(half + 1) * NHALF],
                        av[0:64, :], rs[:],
                    )

        # phase 3: projection, PSUM accumulates the 3 heads
        with tc.tile_pool(name="pj_ps", bufs=2, space="PSUM") as pj_ps:
            for ni in range(NT):
                pp = pj_ps.tile([128, 2, 512], F32, tag="pp")
                for h in range(HPC):
                    for oc in range(2):
                        nc.tensor.matmul(
                            pp[:, oc, 0:384],
                            outTn_t[:, h, ni * 128:(ni + 1) * 128],
                            wp_t[:, h, oc * 384:(oc + 1) * 384],
                            start=(h == 0), stop=(h == HPC - 1),
                        )
                os_t = stg.tile([128, DIM], F32, tag="os")
                nc.scalar.copy(os_t[:, 0:384], pp[:, 0, 0:384])
                nc.scalar.copy(os_t[:, 384:768], pp[:, 1, 0:384])
                nc.sync.dma_start(out_d[ni * 128:(ni + 1) * 128, :], os_t[:])

    nc.compile()
    return nc


def build_in_maps(x, k, v, w_qkv, w_proj):
    x = np.asarray(x, dtype=np.float32)
    k = np.asarray(k, dtype=np.float32)
    v = np.asarray(v, dtype=np.float32)
    wqT = np.ascontiguousarray(np.asarray(w_qkv, np.float32).T)   # [C, 768]
    wpT = np.ascontiguousarray(np.asarray(w_proj, np.float32).T)  # [768, 768]

    in_maps = []
    for core in range(NCORES):
        b = core // 4
        hs = [3 * (core % 4) + i for i in range(HPC)]
        xT = np.ascontiguousarray(x[b].T)                        # [DIM, N]
        wq = np.stack([wqT[:, 64 * h:64 * (h + 1)] for h in hs])  # [3, DIM, D]
        kT = np.ascontiguousarray(k[b, hs].transpose(0, 2, 1))   # [3, D, M]
        va = np.ones((HPC, M, 128), dtype=np.float32)
        va[:, :, :D] = v[b, hs]                                  # [3, M, 128]
        wp = np.stack([wpT[64 * h:64 * (h + 1), :] for h in hs])  # [3, D, DIM]
        in_maps.append({"xT": xT, "wq": wq, "kT": kT,
                        "va": np.ascontiguousarray(va),
                        "wp": np.ascontiguousarray(wp)})
    return in_maps


def kernel(x, k, v, w_qkv, w_proj, b_proj):
    b_proj = np.asarray(b_proj, dtype=np.float32)

    if "nc" not in _cached:
        _cached["nc"] = build_program()
    nc = _cached["nc"]

    in_maps = build_in_maps(x, k, v, w_qkv, w_proj)
    res = run_bass_kernel_spmd(nc, in_maps, core_ids=list(range(NCORES)))

    out = np.empty((B, N, DIM), dtype=np.float32)
    for b in range(B):
        acc = np.zeros((N, DIM), dtype=np.float64)
        for core in range(4 * b, 4 * b + 4):
            acc += res.results[core]["out"]
        out[b] = (acc + b_proj).astype(np.float32)
    return out


# revision 9
# speedup vs baseline: 1.1548x; 1.1235x over previous
"""Multi-head attention (GAttention) on 8 trn2 NeuronCores.

Reference computation (per batch b):
    q = x @ w_qkv.T            -> [N, 768], heads of 64
    attn = softmax(q k^T / 8)  -> per head [N, M]
    out_h = attn @ v           -> [N, 64]
    out = concat(out_h) @ w_proj.T + b_proj

Sharding: 24 (b, head) units over 8 cores -> each core gets one batch b and
3 heads. Each core computes its heads' attention plus its partial
projection sum [N, 768]; host adds the 4 partials per batch + bias.

Per-core device pipeline (all matmuls f32r: 1 cyc/row, ~2.4e-4 rel rounding):
  1. qproj: qT[h] = wq_h^T-slices^T @ x^T        (contraction over C=768)
  2. per head, per key-tile mi (128 keys), per n-half (1024 queries):
       S^T[m, n] = k_h q^T (contraction d=64)    -> PSUM [128, 1024]
       expT = exp(0.125 * S^T)                   -> SBUF f32r (ACT, fused scale)
       AV: outT[128, n] += v_aug[m,128]^T expT   -> PSUM accumulate over mi
     where v_aug = [v_h | ones*64] so PSUM rows 64:128 hold the softmax
     denominator replicated; rows 0:64 hold unnormalized out^T.
  3. normalize: outTn = outT[0:64] * recip(outT[64:128])  (DVE)
  4. proj: out[n-tile, 768] = sum_h outTn_h^T @ wp_h      (PSUM accum over h)
"""
import numpy as np
import ml_dtypes
from contextlib import ExitStack

import concourse.bass as bass
import concourse.mybir as mybir
import concourse.tile as tile
from concourse import bacc
from concourse.bass_utils import run_bass_kernel_spmd

B, N, DIM = 2, 2048, 768
H, D = 12, 64
M = 2048
NCORES = 8
HPC = 3            # heads per core
NT = N // 128      # 16 query tiles
MT = M // 128      # 16 key tiles
CT = DIM // 128    # 6 contraction tiles for qproj
NHALF = 1024       # exp/psum n-granularity
F32 = mybir.dt.float32
F32R = mybir.dt.float32r
BF16 = mybir.dt.bfloat16

_cached = {}


def build_program():
    nc = bacc.Bacc("TRN2", target_bir_lowering=False, debug=False)
    xT_d = nc.dram_tensor("xT", [DIM, N], F32R, kind="ExternalInput")
    wq_d = nc.dram_tensor("wq", [HPC, DIM, D], F32R, kind="ExternalInput")
    kT_d = nc.dram_tensor("kT", [HPC, D, M], BF16, kind="ExternalInput")
    va_d = nc.dram_tensor("va", [HPC, M, 128], BF16, kind="ExternalInput")
    wp_d = nc.dram_tensor("wp", [HPC, D, DIM], F32R, kind="ExternalInput")
    out_d = nc.dram_tensor("out", [N, DIM], F32, kind="ExternalOutput")

    with tile.TileContext(nc) as tc, ExitStack() as ctx:
        big = ctx.enter_context(tc.tile_pool(name="big", bufs=1))
        expp = ctx.enter_context(tc.tile_pool(name="expp", bufs=3))
        stg = ctx.enter_context(tc.tile_pool(name="stg", bufs=3))

        # persistent SBUF tensors
        xT_t = [big.tile([128, N], F32R, name=f"xT{c}", tag=f"xT{c}")
                for c in range(CT)]
        for c in range(CT):
            nc.sync.dma_start(xT_t[c][:], xT_d[c * 128:(c + 1) * 128, :])
        wq_t = big.tile([128, HPC, CT, D], F32R)
        nc.sync.dma_start(
            wq_t[:], wq_d.rearrange("h (c p) d -> p h c d", p=128))
        kT_t = big.tile([64, HPC, M], BF16)
        nc.sync.dma_start(kT_t[:], kT_d.rearrange("h d m -> d h m"))
        va_t = big.tile([128, HPC, MT, 128], BF16)
        nc.sync.dma_start(
            va_t[:], va_d.rearrange("h (t p) e -> p h t e", p=128))
        wp_t = big.tile([64, HPC, DIM], F32R)
        nc.sync.dma_start(wp_t[:], wp_d.rearrange("h d o -> d h o"))
        qT_t = big.tile([64, HPC, N], BF16)
        outTn_t = big.tile([64, HPC, N], F32R)

        # phase 1: q projection, per head: qT[64, N] with PSUM accum over C
        with tc.tile_pool(name="qp_ps", bufs=1, space="PSUM") as qp_ps:
            for h in range(HPC):
                qp = qp_ps.tile([64, N], F32)
                for c in range(CT):
                    for ch in range(N // 512):
                        nc.tensor.matmul(
                            qp[:, ch * 512:(ch + 1) * 512],
                            wq_t[:, h, c, :],
                            xT_t[c][:, ch * 512:(ch + 1) * 512],
                            start=(c == 0), stop=(c == CT - 1),
                        )
                nc.vector.tensor_copy(qT_t[:, h, :], qp[:])

        # phase 2: attention in 6 (head, n-half) units, pipelined via bufs=2
        with tc.tile_pool(name="st_ps", bufs=2, space="PSUM") as st_ps, \
             tc.tile_pool(name="av_ps", bufs=2, space="PSUM") as av_ps:
            for h in range(HPC):
                for half in range(N // NHALF):
                    av = av_ps.tile([128, NHALF], F32, tag="av")
                    for mi in range(MT):
                        kslice = kT_t[:, h, mi * 128:(mi + 1) * 128]
                        st = st_ps.tile([128, NHALF], F32, tag="st")
                        for cc in range(NHALF // 512):
                            n0 = half * NHALF + cc * 512
                            nc.tensor.matmul(
                                st[:, cc * 512:(cc + 1) * 512],
                                kslice,
                                qT_t[:, h, n0:n0 + 512],
                                start=True, stop=True,
                            )
                        et = expp.tile([128, NHALF], BF16, tag="et")
                        nc.scalar.activation(
                            et[:], st[:],
                            mybir.ActivationFunctionType.Exp,
                            scale=float(D) ** -0.5,
                        )
                        for cc in range(NHALF // 512):
                            nc.tensor.matmul(
                                av[:, cc * 512:(cc + 1) * 512],
                                va_t[:, h, mi, :],
                                et[:, cc * 512:(cc + 1) * 512],
                                start=(mi == 0), stop=(mi == MT - 1),
                            )
                    # rows 64:128 of av hold the softmax denominator
                    rs = expp.tile([64, NHALF], F32, tag="rs")
                    nc.vector.reciprocal(rs[:], av[64:128, :])
                    nc.vector.tensor_mul(
                        outTn_t[:, h, half * NHALF:(half + 1) * NHALF],
                        av[0:64, :], rs[:],
                    )

        # phase 3: projection, PSUM accumulates the 3 heads
        with tc.tile_pool(name="pj_ps", bufs=2, space="PSUM") as pj_ps:
            for ni in range(NT):
                pp = pj_ps.tile([128, 2, 512], F32, tag="pp")
                for h in range(HPC):
                    for oc in range(2):
                        nc.tensor.matmul(
                            pp[:, oc, 0:384],
                            outTn_t[:, h, ni * 128:(ni + 1) * 128],
                            wp_t[:, h, oc * 384:(oc + 1) * 384],
                            start=(h == 0), stop=(h == HPC - 1),
                        )
                os_t = stg.tile([128, DIM], F32, tag="os")
                nc.scalar.copy(os_t[:, 0:384], pp[:, 0, 0:384])
                nc.scalar.copy(os_t[:, 384:768], pp[:, 1, 0:384])
                nc.sync.dma_start(out_d[ni * 128:(ni + 1) * 128, :], os_t[:])

    nc.compile()
    return nc


def build_in_maps(x, k, v, w_qkv, w_proj):
    x = np.asarray(x, dtype=np.float32)
    k = np.asarray(k, dtype=np.float32)
    v = np.asarray(v, dtype=np.float32)
    wqT = np.ascontiguousarray(np.asarray(w_qkv, np.float32).T)   # [C, 768]
    wpT = np.ascontiguousarray(np.asarray(w_proj, np.float32).T)  # [768, 768]

    in_maps = []
    for core in range(NCORES):
        b = core // 4
        hs = [3 * (core % 4) + i for i in range(HPC)]
        xT = np.ascontiguousarray(x[b].T)                        # [DIM, N]
        wq = np.stack([wqT[:, 64 * h:64 * (h + 1)] for h in hs])  # [3, DIM, D]
        kT = np.ascontiguousarray(
            k[b, hs].transpose(0, 2, 1).astype(ml_dtypes.bfloat16))
        va = np.ones((HPC, M, 128), dtype=ml_dtypes.bfloat16)
        va[:, :, :D] = v[b, hs].astype(ml_dtypes.bfloat16)       # [3, M, 128]
        wp = np.stack([wpT[64 * h:64 * (h + 1), :] for h in hs])  # [3, D, DIM]
        in_maps.append({"xT": xT, "wq": wq, "kT": kT,
                        "va": np.ascontiguousarray(va),
                        "wp": np.ascontiguousarray(wp)})
    return in_maps


def kernel(x, k, v, w_qkv, w_proj, b_proj):
    b_proj = np.asarray(b_proj, dtype=np.float32)

    if "nc" not in _cached:
        _cached["nc"] = build_program()
    nc = _cached["nc"]

    in_maps = build_in_maps(x, k, v, w_qkv, w_proj)
    res = run_bass_kernel_spmd(nc, in_maps, core_ids=list(range(NCORES)))

    out = np.empty((B, N, DIM), dtype=np.float32)
    for b in range(B):
        acc = np.zeros((N, DIM), dtype=np.float64)
        for core in range(4 * b, 4 * b + 4):
            acc += res.results[core]["out"]
        out[b] = (acc + b_proj).astype(np.float32)
    return out


# revision 10
# speedup vs baseline: 1.5792x; 1.3675x over previous
"""Multi-head attention (GAttention) on 8 trn2 NeuronCores.

Reference computation (per batch b):
    q = x @ w_qkv.T            -> [N, 768], heads of 64
    attn = softmax(q k^T / 8)  -> per head [N, M]
    out_h = attn @ v           -> [N, 64]
    out = concat(out_h) @ w_proj.T + b_proj

Sharding: 24 (b, head) units over 8 cores -> each core gets one batch b and
3 heads. Each core computes its heads' attention plus its partial
projection sum [N, 768]; host adds the 4 partials per batch + bias.

Per-core device pipeline:
  1. qproj (f32r): qT_dup[128, N] per head = [wq_h | wq_h]^T x^T; the
     duplicated column block makes rows 64:128 a copy of rows 0:64, which
     feeds the row-packed S^T matmuls.
  2. attention (bf16 operands, f32 PSUM), 6 (head, n-half) units; per key
     m-tile PAIR (2 x 128 keys, PE row groups 0/64 run concurrently):
       S^T = k q^T   -> PSUM [128, 2, 512] per n-chunk (tile A/B)
       expT = exp(0.125 S^T) -> SBUF bf16 (ACT, fused scale)
       AV: av[128, 1024] += v_aug^T expT   (accumulate over all 16 m-tiles)
     v_aug = [v_h | ones*64] so av rows 64:128 hold the softmax denominator.
  3. normalize: outTn (both partition halves) = av[0:64] * recip(av[64:128])
  4. proj (f32r): row-packed n-tile pairs, PSUM accumulates the 3 heads.
"""
import numpy as np
import ml_dtypes
from contextlib import ExitStack

import concourse.bass as bass
import concourse.mybir as mybir
import concourse.tile as tile
from concourse import bacc
from concourse.bass_utils import run_bass_kernel_spmd

B, N, DIM = 2, 2048, 768
H, D = 12, 64
M = 2048
NCORES = 8
HPC = 3            # heads per core
NT = N // 128      # 16 query tiles
MT = M // 128      # 16 key tiles
MP = MT // 2       # 8 key-tile pairs
CT = DIM // 128    # 6 contraction tiles for qproj
NHALF = 1024       # AV psum n-granularity
F32 = mybir.dt.float32
F32R = mybir.dt.float32r
BF16 = mybir.dt.bfloat16

_cached = {}


def build_program():
    nc = bacc.Bacc("TRN2", target_bir_lowering=False, debug=False)
    xT_d = nc.dram_tensor("xT", [DIM, N], F32R, kind="ExternalInput")
    wq_d = nc.dram_tensor("wq", [HPC, DIM, 128], F32R, kind="ExternalInput")
    kT_d = nc.dram_tensor("kT", [128, HPC, MP, 128], BF16,
                          kind="ExternalInput")
    va_d = nc.dram_tensor("va", [HPC, M, 128], BF16, kind="ExternalInput")
    wp_d = nc.dram_tensor("wp", [128, HPC, DIM], F32R, kind="ExternalInput")
    out_d = nc.dram_tensor("out", [N, DIM], F32, kind="ExternalOutput")

    with tile.TileContext(nc) as tc, ExitStack() as ctx:
        big = ctx.enter_context(tc.tile_pool(name="big", bufs=1))
        expp = ctx.enter_context(tc.tile_pool(name="expp", bufs=4))
        stg = ctx.enter_context(tc.tile_pool(name="stg", bufs=3))

        # persistent SBUF tensors
        xT_t = [big.tile([128, N], F32R, name=f"xT{c}", tag=f"xT{c}")
                for c in range(CT)]
        for c in range(CT):
            nc.sync.dma_start(xT_t[c][:], xT_d[c * 128:(c + 1) * 128, :])
        wq_t = big.tile([128, HPC, CT, 128], F32R)
        nc.sync.dma_start(
            wq_t[:], wq_d.rearrange("h (c p) d -> p h c d", p=128))
        kT_t = big.tile([128, HPC, MP, 128], BF16)
        nc.sync.dma_start(kT_t[:], kT_d[:])
        va_t = big.tile([128, HPC, MT, 128], BF16)
        nc.sync.dma_start(
            va_t[:], va_d.rearrange("h (t p) e -> p h t e", p=128))
        wp_t = big.tile([128, HPC, DIM], F32R)
        nc.sync.dma_start(wp_t[:], wp_d[:])
        qT_t = big.tile([128, HPC, N], BF16)
        outTn_t = big.tile([128, HPC, N], F32R)

        # phase 1: q projection; wq has the head slice duplicated so rows
        # 64:128 of qT_t replicate rows 0:64
        with tc.tile_pool(name="qp_ps", bufs=1, space="PSUM") as qp_ps:
            for h in range(HPC):
                qp = qp_ps.tile([128, N], F32)
                for c in range(CT):
                    for ch in range(N // 512):
                        nc.tensor.matmul(
                            qp[:, ch * 512:(ch + 1) * 512],
                            wq_t[:, h, c, :],
                            xT_t[c][:, ch * 512:(ch + 1) * 512],
                            start=(c == 0), stop=(c == CT - 1),
                        )
                nc.vector.tensor_copy(qT_t[:, h, :], qp[:])

        # phase 2: attention in 6 (head, n-half) units; m-tile pairs are
        # row-packed on the PE (row groups 0 and 64)
        with tc.tile_pool(name="st_ps", bufs=2, space="PSUM") as st_ps, \
             tc.tile_pool(name="av_ps", bufs=2, space="PSUM") as av_ps:
            for h in range(HPC):
                for half in range(N // NHALF):
                    av = av_ps.tile([128, NHALF], F32, tag="av")
                    for p in range(MP):
                        for cc in range(NHALF // 512):
                            n0 = half * NHALF + cc * 512
                            st = st_ps.tile([128, 2, 512], F32, tag="st")
                            nc.tensor.matmul(
                                st[:, 0, :], kT_t[0:64, h, p, :],
                                qT_t[0:64, h, n0:n0 + 512],
                                start=True, stop=True, tile_position=(0, 0),
                            )
                            nc.tensor.matmul(
                                st[:, 1, :], kT_t[64:128, h, p, :],
                                qT_t[64:128, h, n0:n0 + 512],
                                start=True, stop=True, tile_position=(64, 0),
                            )
                            et = expp.tile([128, 2, 512], BF16, tag="et")
                            nc.scalar.activation(
                                et[:], st[:],
                                mybir.ActivationFunctionType.Exp,
                                scale=float(D) ** -0.5,
                            )
                            first = (p == 0)
                            last = (p == MP - 1)
                            nc.tensor.matmul(
                                av[:, cc * 512:(cc + 1) * 512],
                                va_t[:, h, 2 * p, :], et[:, 0, :],
                                start=first, stop=False,
                            )
                            nc.tensor.matmul(
                                av[:, cc * 512:(cc + 1) * 512],
                                va_t[:, h, 2 * p + 1, :], et[:, 1, :],
                                start=False, stop=last,
                            )
                    # rows 64:128 of av hold the softmax denominator;
                    # write both partition halves of outTn for proj packing
                    rs = expp.tile([64, NHALF], F32, tag="rs")
                    nc.vector.reciprocal(rs[:], av[64:128, :])
                    nsl = slice(half * NHALF, (half + 1) * NHALF)
                    nc.vector.tensor_mul(
                        outTn_t[0:64, h, nsl], av[0:64, :], rs[:])
                    nc.vector.tensor_mul(
                        outTn_t[64:128, h, nsl], av[0:64, :], rs[:])

        # phase 3: projection, row-packed n-tile pairs, PSUM accumulates
        # the 3 heads
        with tc.tile_pool(name="pj_ps", bufs=2, space="PSUM") as pj_ps:
            for nj in range(NT // 2):
                ppa = pj_ps.tile([128, 2, 512], F32, tag="ppa")
                ppb = pj_ps.tile([128, 2, 512], F32, tag="ppb")
                na = 2 * nj * 128
                nb = (2 * nj + 1) * 128
                for h in range(HPC):
                    for oc in range(2):
                        osl = slice(oc * 384, (oc + 1) * 384)
                        nc.tensor.matmul(
                            ppa[:, oc, 0:384],
                            outTn_t[0:64, h, na:na + 128],
                            wp_t[0:64, h, osl],
                            start=(h == 0), stop=(h == HPC - 1),
                            tile_position=(0, 0),
                        )
                        nc.tensor.matmul(
                            ppb[:, oc, 0:384],
                            outTn_t[64:128, h, nb:nb + 128],
                            wp_t[64:128, h, osl],
                            start=(h == 0), stop=(h == HPC - 1),
                            tile_position=(64, 0),
                        )
                for which, pp, nn in ((0, ppa, na), (1, ppb, nb)):
                    os_t = stg.tile([128, DIM], F32, tag="os")
                    nc.scalar.copy(os_t[:, 0:384], pp[:, 0, 0:384])
                    nc.scalar.copy(os_t[:, 384:768], pp[:, 1, 0:384])
                    nc.sync.dma_start(out_d[nn:nn + 128, :], os_t[:])

    nc.compile()
    return nc


def build_in_maps(x, k, v, w_qkv, w_proj):
    x = np.asarray(x, dtype=np.float32)
    k = np.asarray(k, dtype=np.float32)
    v = np.asarray(v, dtype=np.float32)
    wqT = np.ascontiguousarray(np.asarray(w_qkv, np.float32).T)   # [C, 768]
    wpT = np.ascontiguousarray(np.asarray(w_proj, np.float32).T)  # [768, 768]

    in_maps = []
    for core in range(NCORES):
        b = core // 4
        hs = [3 * (core % 4) + i for i in range(HPC)]
        xT = np.ascontiguousarray(x[b].T)                        # [DIM, N]
        # duplicated head slice -> qT rows 64:128 == rows 0:64
        wq = np.stack([
            np.concatenate([wqT[:, 64 * h:64 * (h + 1)]] * 2, axis=1)
            for h in hs])                                        # [3, DIM, 128]
        # kT layout [128, HPC, MP, 128]: rows 0:64 = head-dim of even m-tile,
        # rows 64:128 = head-dim of odd m-tile of each pair
        kb = k[b, hs].astype(ml_dtypes.bfloat16)                 # [3, M, D]
        kT = np.empty((128, HPC, MP, 128), dtype=ml_dtypes.bfloat16)
        for hi in range(HPC):
            for p in range(MP):
                kT[0:64, hi, p, :] = kb[hi, 256 * p:256 * p + 128, :].T
                kT[64:128, hi, p, :] = kb[hi, 256 * p + 128:256 * p + 256, :].T
        va = np.ones((HPC, M, 128), dtype=ml_dtypes.bfloat16)
        va[:, :, :D] = v[b, hs].astype(ml_dtypes.bfloat16)       # [3, M, 128]
        # wp duplicated on both partition halves for row-packed proj
        wp = np.empty((128, HPC, DIM), dtype=np.float32)
        for hi, h in enumerate(hs):
            wp[0:64, hi, :] = wpT[64 * h:64 * (h + 1), :]
            wp[64:128, hi, :] = wpT[64 * h:64 * (h + 1), :]
        in_maps.append({"xT": xT, "wq": wq,
                        "kT": np.ascontiguousarray(kT),
                        "va": np.ascontiguousarray(va),
                        "wp": np.ascontiguousarray(wp)})
    return in_maps


def kernel(x, k, v, w_qkv, w_proj, b_proj):
    b_proj = np.asarray(b_proj, dtype=np.float32)

    if "nc" not in _cached:
        _cached["nc"] = build_program()
    nc = _cached["nc"]

    in_maps = build_in_maps(x, k, v, w_qkv, w_proj)
    res = run_bass_kernel_spmd(nc, in_maps, core_ids=list(range(NCORES)))

    out = np.empty((B, N, DIM), dtype=np.float32)
    for b in range(B):
        acc = np.zeros((N, DIM), dtype=np.float64)
        for core in range(4 * b, 4 * b + 4):
            acc += res.results[core]["out"]
        out[b] = (acc + b_proj).astype(np.float32)
    return out


# revision 15
# speedup vs baseline: 1.5810x; 1.0011x over previous
"""Multi-head attention (GAttention) on 8 trn2 NeuronCores.

Reference computation (per batch b):
    q = x @ w_qkv.T            -> [N, 768], heads of 64
    attn = softmax(q k^T / 8)  -> per head [N, M]
    out_h = attn @ v           -> [N, 64]
    out = concat(out_h) @ w_proj.T + b_proj

Sharding: 24 (b, head) units over 8 cores -> each core gets one batch b and
3 heads. Each core computes its heads' attention plus its partial
projection sum [N, 768]; host adds the 4 partials per batch + bias.

Per-core device pipeline:
  1. qproj (f32r): qT_dup[128, N] per head = [wq_h | wq_h]^T x^T; the
     duplicated column block makes rows 64:128 a copy of rows 0:64, which
     feeds the row-packed S^T matmuls.
  2. attention (bf16 operands, f32 PSUM), 6 (head, n-half) units; per key
     m-tile PAIR (2 x 128 keys, PE row groups 0/64 run concurrently):
       S^T = k q^T   -> PSUM [128, 2, 512] per n-chunk (tile A/B)
       expT = exp(0.125 S^T) -> SBUF bf16 (ACT, fused scale)
       AV: av[128, 1024] += v_aug^T expT   (accumulate over all 16 m-tiles)
     v_aug = [v_h | ones*64] so av rows 64:128 hold the softmax denominator.
  3. normalize: outTn (both partition halves) = av[0:64] * recip(av[64:128])
  4. proj (f32r): row-packed n-tile pairs, PSUM accumulates the 3 heads.
"""
import numpy as np
import ml_dtypes
from contextlib import ExitStack

import concourse.bass as bass
import concourse.mybir as mybir
import concourse.tile as tile
from concourse import bacc
from concourse.bass_utils import run_bass_kernel_spmd

B, N, DIM = 2, 2048, 768
H, D = 12, 64
M = 2048
NCORES = 8
HPC = 3            # heads per core
NT = N // 128      # 16 query tiles
MT = M // 128      # 16 key tiles
MP = MT // 2       # 8 key-tile pairs
CT = DIM // 128    # 6 contraction tiles for qproj
NHALF = 1024       # AV psum n-granularity
F32 = mybir.dt.float32
F32R = mybir.dt.float32r
BF16 = mybir.dt.bfloat16

_cached = {}


def build_program():
    nc = bacc.Bacc("TRN2", target_bir_lowering=False, debug=False)
    xT_d = nc.dram_tensor("xT", [DIM, N], F32R, kind="ExternalInput")
    wq_d = nc.dram_tensor("wq", [HPC, DIM, 128], F32R, kind="ExternalInput")
    kT_d = nc.dram_tensor("kT", [128, HPC, MP, 128], BF16,
                          kind="ExternalInput")
    va_d = nc.dram_tensor("va", [HPC, M, 128], BF16, kind="ExternalInput")
    wp_d = nc.dram_tensor("wp", [128, HPC, DIM], F32R, kind="ExternalInput")
    out_d = nc.dram_tensor("out", [N, DIM], F32, kind="ExternalOutput")

    with tile.TileContext(nc) as tc, ExitStack() as ctx:
        big = ctx.enter_context(tc.tile_pool(name="big", bufs=1))
        expp = ctx.enter_context(tc.tile_pool(name="expp", bufs=4))
        stg = ctx.enter_context(tc.tile_pool(name="stg", bufs=3))

        # persistent SBUF tensors
        xT_t = [big.tile([128, N], F32R, name=f"xT{c}", tag=f"xT{c}")
                for c in range(CT)]
        for c in range(CT):
            nc.sync.dma_start(xT_t[c][:], xT_d[c * 128:(c + 1) * 128, :])
        wq_t = big.tile([128, HPC, CT, 128], F32R)
        nc.sync.dma_start(
            wq_t[:], wq_d.rearrange("h (c p) d -> p h c d", p=128))
        kT_t = big.tile([128, HPC, MP, 128], BF16)
        nc.sync.dma_start(kT_t[:], kT_d[:])
        va_t = big.tile([128, HPC, MT, 128], BF16)
        nc.sync.dma_start(
            va_t[:], va_d.rearrange("h (t p) e -> p h t e", p=128))
        wp_t = big.tile([128, HPC, DIM], F32R)
        nc.sync.dma_start(wp_t[:], wp_d[:])
        qT_t = big.tile([128, HPC, N], BF16)
        outTn_t = big.tile([128, HPC, N], F32R)

        # phase 1: q projection; wq has the head slice duplicated so rows
        # 64:128 of qT_t replicate rows 0:64
        with tc.tile_pool(name="qp_ps", bufs=1, space="PSUM") as qp_ps:
            for h in range(HPC):
                qp = qp_ps.tile([128, N], F32)
                for c in range(CT):
                    for ch in range(N // 512):
                        nc.tensor.matmul(
                            qp[:, ch * 512:(ch + 1) * 512],
                            wq_t[:, h, c, :],
                            xT_t[c][:, ch * 512:(ch + 1) * 512],
                            start=(c == 0), stop=(c == CT - 1),
                        )
                nc.vector.tensor_copy(qT_t[:, h, :], qp[:])

        # phase 2: attention in 6 (head, n-half) units; m-tile pairs are
        # row-packed on the PE (row groups 0 and 64). The AV matmuls for
        # iteration i are issued AFTER iteration i+1's S^T so the in-order
        # PE queue never stalls behind the EXP wait.
        with tc.tile_pool(name="st_ps", bufs=2, space="PSUM") as st_ps, \
             tc.tile_pool(name="av_ps", bufs=2, space="PSUM") as av_ps:
            av_by_unit = {}

            def _av(pend):
                unit, et, p, cc, first, last = pend
                av = av_by_unit[unit]
                nc.tensor.matmul(
                    av[:, cc * 512:(cc + 1) * 512],
                    va_t[:, unit[0], 2 * p, :], et[:, 0, :],
                    start=first, stop=False,
                )
                nc.tensor.matmul(
                    av[:, cc * 512:(cc + 1) * 512],
                    va_t[:, unit[0], 2 * p + 1, :], et[:, 1, :],
                    start=False, stop=last,
                )

            def _norm(unit):
                h, half = unit
                av = av_by_unit[unit]
                rs = expp.tile([64, NHALF], F32, tag="rs", name="rs")
                nc.vector.reciprocal(rs[:], av[64:128, :])
                nsl = slice(half * NHALF, (half + 1) * NHALF)
                nc.vector.tensor_mul(
                    outTn_t[0:64, h, nsl], av[0:64, :], rs[:])
                nc.vector.tensor_mul(
                    outTn_t[64:128, h, nsl], av[0:64, :], rs[:])

            iters = [(h, half, p, cc)
                     for h in range(HPC) for half in range(N // NHALF)
                     for p in range(MP) for cc in range(NHALF // 512)]
            pend = None
            for h, half, p, cc in iters:
                unit = (h, half)
                if unit not in av_by_unit:
                    av_by_unit[unit] = av_ps.tile(
                        [128, NHALF], F32, tag="av", name="av")
                n0 = half * NHALF + cc * 512
                st = st_ps.tile([128, 2, 512], F32, tag="st", name="st")
                nc.tensor.matmul(
                    st[:, 0, :], kT_t[0:64, h, p, :],
                    qT_t[0:64, h, n0:n0 + 512],
                    start=True, stop=True, tile_position=(0, 0),
                )
                nc.tensor.matmul(
                    st[:, 1, :], kT_t[64:128, h, p, :],
                    qT_t[64:128, h, n0:n0 + 512],
                    start=True, stop=True, tile_position=(64, 0),
                )
                if pend is not None:
                    _av(pend)
                    if pend[0] != unit:
                        _norm(pend[0])
                et = expp.tile([128, 2, 512], BF16, tag="et", name="et")
                nc.scalar.activation(
                    et[:], st[:], mybir.ActivationFunctionType.Exp,
                    scale=float(D) ** -0.5,
                )
                pend = (unit, et, p, cc, p == 0, p == MP - 1)
            _av(pend)
            _norm(pend[0])

        # phase 3: projection, row-packed n-tile pairs, PSUM accumulates
        # the 3 heads
        with tc.tile_pool(name="pj_ps", bufs=2, space="PSUM") as pj_ps:
            for nj in range(NT // 2):
                ppa = pj_ps.tile([128, 2, 512], F32, tag="ppa")
                ppb = pj_ps.tile([128, 2, 512], F32, tag="ppb")
                na = 2 * nj * 128
                nb = (2 * nj + 1) * 128
                for h in range(HPC):
                    for oc in range(2):
                        osl = slice(oc * 384, (oc + 1) * 384)
                        nc.tensor.matmul(
                            ppa[:, oc, 0:384],
                            outTn_t[0:64, h, na:na + 128],
                            wp_t[0:64, h, osl],
                            start=(h == 0), stop=(h == HPC - 1),
                            tile_position=(0, 0),
                        )
                        nc.tensor.matmul(
                            ppb[:, oc, 0:384],
                            outTn_t[64:128, h, nb:nb + 128],
                            wp_t[64:128, h, osl],
                            start=(h == 0), stop=(h == HPC - 1),
                            tile_position=(64, 0),
                        )
                for which, pp, nn in ((0, ppa, na), (1, ppb, nb)):
                    os_t = stg.tile([128, DIM], F32, tag="os", name="os")
                    nc.vector.tensor_copy(os_t[:, 0:384], pp[:, 0, 0:384])
                    nc.vector.tensor_copy(os_t[:, 384:768], pp[:, 1, 0:384])
                    nc.sync.dma_start(out_d[nn:nn + 128, :], os_t[:])

    nc.compile()
    return nc


def build_in_maps(x, k, v, w_qkv, w_proj):
    x = np.asarray(x, dtype=np.float32)
    k = np.asarray(k, dtype=np.float32)
    v = np.asarray(v, dtype=np.float32)
    wqT = np.ascontiguousarray(np.asarray(w_qkv, np.float32).T)   # [C, 768]
    wpT = np.ascontiguousarray(np.asarray(w_proj, np.float32).T)  # [768, 768]

    in_maps = []
    for core in range(NCORES):
        b = core // 4
        hs = [3 * (core % 4) + i for i in range(HPC)]
        xT = np.ascontiguousarray(x[b].T)                        # [DIM, N]
        # duplicated head slice -> qT rows 64:128 == rows 0:64
        wq = np.stack([
            np.concatenate([wqT[:, 64 * h:64 * (h + 1)]] * 2, axis=1)
            for h in hs])                                        # [3, DIM, 128]
        # kT layout [128, HPC, MP, 128]: rows 0:64 = head-dim of even m-tile,
        # rows 64:128 = head-dim of odd m-tile of each pair
        kb = k[b, hs].astype(ml_dtypes.bfloat16)                 # [3, M, D]
        kT = np.empty((128, HPC, MP, 128), dtype=ml_dtypes.bfloat16)
        for hi in range(HPC):
            for p in range(MP):
                kT[0:64, hi, p, :] = kb[hi, 256 * p:256 * p + 128, :].T
                kT[64:128, hi, p, :] = kb[hi, 256 * p + 128:256 * p + 256, :].T
        va = np.ones((HPC, M, 128), dtype=ml_dtypes.bfloat16)
        va[:, :, :D] = v[b, hs].astype(ml_dtypes.bfloat16)       # [3, M, 128]
        # wp duplicated on both partition halves for row-packed proj
        wp = np.empty((128, HPC, DIM), dtype=np.float32)
        for hi, h in enumerate(hs):
            wp[0:64, hi, :] = wpT[64 * h:64 * (h + 1), :]
            wp[64:128, hi, :] = wpT[64 * h:64 * (h + 1), :]
        in_maps.append({"xT": xT, "wq": wq,
                        "kT": np.ascontiguousarray(kT),
                        "va": np.ascontiguousarray(va),
                        "wp": np.ascontiguousarray(wp)})
    return in_maps


def kernel(x, k, v, w_qkv, w_proj, b_proj):
    b_proj = np.asarray(b_proj, dtype=np.float32)

    if "nc" not in _cached:
        _cached["nc"] = build_program()
    nc = _cached["nc"]

    in_maps = build_in_maps(x, k, v, w_qkv, w_proj)
    res = run_bass_kernel_spmd(nc, in_maps, core_ids=list(range(NCORES)))

    out = np.empty((B, N, DIM), dtype=np.float32)
    for b in range(B):
        acc = np.zeros((N, DIM), dtype=np.float64)
        for core in range(4 * b, 4 * b + 4):
            acc += res.results[core]["out"]
        out[b] = (acc + b_proj).astype(np.float32)
    return out


# revision 17
# speedup vs baseline: 1.7937x; 1.1345x over previous
"""Multi-head attention (GAttention) on 8 trn2 NeuronCores.

Reference computation (per batch b):
    q = x @ w_qkv.T            -> [N, 768], heads of 64
    attn = softmax(q k^T / 8)  -> per head [N, M]
    out_h = attn @ v           -> [N, 64]
    out = concat(out_h) @ w_proj.T + b_proj

Sharding: 24 (b, head) units over 8 cores -> each core gets one batch b and
3 heads. Each core computes its heads' attention plus its partial
projection sum [N, 768]; host adds the 4 partials per batch + bias.

Per-core device pipeline:
  1. qproj (f32r): qT_dup[128, N] per head = [wq_h | wq_h]^T x^T; the
     duplicated column block makes rows 64:128 a copy of rows 0:64, which
     feeds the row-packed S^T matmuls.
  2. attention (bf16 operands, f32 PSUM), 6 (head, n-half) units; per key
     m-tile PAIR (2 x 128 keys, PE row groups 0/64 run concurrently):
       S^T = k q^T   -> PSUM [128, 2, 512] per n-chunk (tile A/B)
       expT = exp(0.125 S^T) -> SBUF bf16 (ACT, fused scale)
       AV: av[128, 1024] += v_aug^T expT   (accumulate over all 16 m-tiles)
     v_aug = [v_h | ones*64] so av rows 64:128 hold the softmax denominator.
  3. normalize: outTn (both partition halves) = av[0:64] * recip(av[64:128])
  4. proj (f32r): row-packed n-tile pairs, PSUM accumulates the 3 heads.
"""
import numpy as np
import ml_dtypes
from contextlib import ExitStack

import concourse.bass as bass
import concourse.mybir as mybir
import concourse.tile as tile
from concourse import bacc
from concourse.bass_utils import run_bass_kernel_spmd

B, N, DIM = 2, 2048, 768
H, D = 12, 64
M = 2048
NCORES = 8
HPC = 3            # heads per core
NT = N // 128      # 16 query tiles
MT = M // 128      # 16 key tiles
MP = MT // 2       # 8 key-tile pairs
CT = DIM // 128    # 6 contraction tiles for qproj
NHALF = 1024       # AV psum n-granularity
F32 = mybir.dt.float32
F32R = mybir.dt.float32r
BF16 = mybir.dt.bfloat16

_cached = {}


def build_program():
    nc = bacc.Bacc("TRN2", target_bir_lowering=False, debug=False)
    xT_d = nc.dram_tensor("xT", [DIM, N], BF16, kind="ExternalInput")
    wq_d = nc.dram_tensor("wq", [HPC, DIM, 128], BF16, kind="ExternalInput")
    kT_d = nc.dram_tensor("kT", [128, HPC, MP, 128], BF16,
                          kind="ExternalInput")
    va_d = nc.dram_tensor("va", [HPC, M, 128], BF16, kind="ExternalInput")
    wp_d = nc.dram_tensor("wp", [128, HPC, DIM], F32R, kind="ExternalInput")
    out_d = nc.dram_tensor("out", [N, DIM], F32, kind="ExternalOutput")

    with tile.TileContext(nc) as tc, ExitStack() as ctx:
        big = ctx.enter_context(tc.tile_pool(name="big", bufs=1))
        expp = ctx.enter_context(tc.tile_pool(name="expp", bufs=4))
        stg = ctx.enter_context(tc.tile_pool(name="stg", bufs=3))

        # persistent SBUF tensors
        xT_t = [big.tile([128, N], BF16, name=f"xT{c}", tag=f"xT{c}")
                for c in range(CT)]
        for c in range(CT):
            nc.sync.dma_start(xT_t[c][:], xT_d[c * 128:(c + 1) * 128, :])
        wq_t = big.tile([128, HPC, CT, 128], BF16)
        nc.sync.dma_start(
            wq_t[:], wq_d.rearrange("h (c p) d -> p h c d", p=128))
        kT_t = big.tile([128, HPC, MP, 128], BF16)
        nc.sync.dma_start(kT_t[:], kT_d[:])
        va_t = big.tile([128, HPC, MT, 128], BF16)
        nc.sync.dma_start(
            va_t[:], va_d.rearrange("h (t p) e -> p h t e", p=128))
        wp_t = big.tile([128, HPC, DIM], F32R)
        nc.sync.dma_start(wp_t[:], wp_d[:])
        qT_t = big.tile([128, HPC, N], BF16)
        outTn_t = big.tile([128, HPC, N], F32R)

        # phase 1: q projection; wq has the head slice duplicated so rows
        # 64:128 of qT_t replicate rows 0:64
        with tc.tile_pool(name="qp_ps", bufs=1, space="PSUM") as qp_ps:
            for h in range(HPC):
                qp = qp_ps.tile([128, N], F32)
                for c in range(CT):
                    for ch in range(N // 512):
                        nc.tensor.matmul(
                            qp[:, ch * 512:(ch + 1) * 512],
                            wq_t[:, h, c, :],
                            xT_t[c][:, ch * 512:(ch + 1) * 512],
                            start=(c == 0), stop=(c == CT - 1),
                        )
                nc.vector.tensor_copy(qT_t[:, h, :], qp[:])

        # phase 2: attention in 6 (head, n-half) units; m-tile pairs are
        # row-packed on the PE (row groups 0 and 64). The AV matmuls for
        # iteration i are issued AFTER iteration i+1's S^T so the in-order
        # PE queue never stalls behind the EXP wait.
        with tc.tile_pool(name="st_ps", bufs=2, space="PSUM") as st_ps, \
             tc.tile_pool(name="av_ps", bufs=2, space="PSUM") as av_ps:
            av_by_unit = {}

            def _av(pend):
                unit, et, p, cc, first, last = pend[:6]
                av = av_by_unit[unit]
                nc.tensor.matmul(
                    av[:, cc * 512:(cc + 1) * 512],
                    va_t[:, unit[0], 2 * p, :], et[:, 0, :],
                    start=first, stop=False,
                )
                nc.tensor.matmul(
                    av[:, cc * 512:(cc + 1) * 512],
                    va_t[:, unit[0], 2 * p + 1, :], et[:, 1, :],
                    start=False, stop=last,
                )

            def _norm(unit):
                h, half = unit
                av = av_by_unit[unit]
                dn = expp.tile([64, NHALF], F32, tag="dn", name="dn")
                nc.vector.tensor_copy(dn[:], av[64:128, :])
                rs = expp.tile([64, NHALF], F32, tag="rs", name="rs")
                nc.vector.reciprocal_approx_fast(rs[:], dn[:])
                nsl = slice(half * NHALF, (half + 1) * NHALF)
                nc.vector.tensor_mul(
                    outTn_t[0:64, h, nsl], av[0:64, :], rs[:])
                nc.vector.tensor_mul(
                    outTn_t[64:128, h, nsl], av[0:64, :], rs[:])

            iters = [(h, half, p, cc)
                     for h in range(HPC) for half in range(N // NHALF)
                     for p in range(MP) for cc in range(NHALF // 512)]
            pend = []
            LAG = 2

            def _flush(limit):
                while len(pend) > limit:
                    pd = pend.pop(0)
                    _av(pd)
                    if pd[6]:
                        _norm(pd[0])

            for h, half, p, cc in iters:
                unit = (h, half)
                if unit not in av_by_unit:
                    av_by_unit[unit] = av_ps.tile(
                        [128, NHALF], F32, tag="av", name="av")
                n0 = half * NHALF + cc * 512
                st = st_ps.tile([128, 2, 512], F32, tag="st", name="st")
                nc.tensor.matmul(
                    st[:, 0, :], kT_t[0:64, h, p, :],
                    qT_t[0:64, h, n0:n0 + 512],
                    start=True, stop=True, tile_position=(0, 0),
                )
                nc.tensor.matmul(
                    st[:, 1, :], kT_t[64:128, h, p, :],
                    qT_t[64:128, h, n0:n0 + 512],
                    start=True, stop=True, tile_position=(64, 0),
                )
                _flush(LAG - 1)
                et = expp.tile([128, 2, 512], BF16, tag="et", name="et")
                nc.scalar.activation(
                    et[:], st[:], mybir.ActivationFunctionType.Exp,
                    scale=float(D) ** -0.5,
                )
                pend.append((unit, et, p, cc, p == 0, p == MP - 1,
                             p == MP - 1 and cc == NHALF // 512 - 1))
            _flush(0)

        # phase 3: projection, row-packed n-tile pairs, PSUM accumulates
        # the 3 heads
        with tc.tile_pool(name="pj_ps", bufs=2, space="PSUM") as pj_ps:
            for nj in range(NT // 2):
                ppa = pj_ps.tile([128, 2, 512], F32, tag="ppa")
                ppb = pj_ps.tile([128, 2, 512], F32, tag="ppb")
                na = 2 * nj * 128
                nb = (2 * nj + 1) * 128
                for h in range(HPC):
                    for oc in range(2):
                        osl = slice(oc * 384, (oc + 1) * 384)
                        nc.tensor.matmul(
                            ppa[:, oc, 0:384],
                            outTn_t[0:64, h, na:na + 128],
                            wp_t[0:64, h, osl],
                            start=(h == 0), stop=(h == HPC - 1),
                            tile_position=(0, 0),
                        )
                        nc.tensor.matmul(
                            ppb[:, oc, 0:384],
                            outTn_t[64:128, h, nb:nb + 128],
                            wp_t[64:128, h, osl],
                            start=(h == 0), stop=(h == HPC - 1),
                            tile_position=(64, 0),
                        )
                for which, pp, nn in ((0, ppa, na), (1, ppb, nb)):
                    os_t = stg.tile([128, DIM], F32, tag="os", name="os")
                    nc.vector.tensor_copy(os_t[:, 0:384], pp[:, 0, 0:384])
                    nc.vector.tensor_copy(os_t[:, 384:768], pp[:, 1, 0:384])
                    nc.sync.dma_start(out_d[nn:nn + 128, :], os_t[:])

    nc.compile()
    return nc


def build_in_maps(x, k, v, w_qkv, w_proj):
    x = np.asarray(x, dtype=np.float32)
    k = np.asarray(k, dtype=np.float32)
    v = np.asarray(v, dtype=np.float32)
    wqT = np.ascontiguousarray(np.asarray(w_qkv, np.float32).T)   # [C, 768]
    wpT = np.ascontiguousarray(np.asarray(w_proj, np.float32).T)  # [768, 768]

    in_maps = []
    for core in range(NCORES):
        b = core // 4
        hs = [3 * (core % 4) + i for i in range(HPC)]
        xT = np.ascontiguousarray(x[b].T.astype(ml_dtypes.bfloat16))
        # duplicated head slice -> qT rows 64:128 == rows 0:64
        wq = np.stack([
            np.concatenate([wqT[:, 64 * h:64 * (h + 1)]] * 2, axis=1)
            for h in hs]).astype(ml_dtypes.bfloat16)             # [3, DIM, 128]
        # kT layout [128, HPC, MP, 128]: rows 0:64 = head-dim of even m-tile,
        # rows 64:128 = head-dim of odd m-tile of each pair
        kb = k[b, hs].astype(ml_dtypes.bfloat16)                 # [3, M, D]
        kT = np.empty((128, HPC, MP, 128), dtype=ml_dtypes.bfloat16)
        for hi in range(HPC):
            for p in range(MP):
                kT[0:64, hi, p, :] = kb[hi, 256 * p:256 * p + 128, :].T
                kT[64:128, hi, p, :] = kb[hi, 256 * p + 128:256 * p + 256, :].T
        va = np.ones((HPC, M, 128), dtype=ml_dtypes.bfloat16)
        va[:, :, :D] = v[b, hs].astype(ml_dtypes.bfloat16)       # [3, M, 128]
        # wp duplicated on both partition halves for row-packed proj
        wp = np.empty((128, HPC, DIM), dtype=np.float32)
        for hi, h in enumerate(hs):
            wp[0:64, hi, :] = wpT[64 * h:64 * (h + 1), :]
            wp[64:128, hi, :] = wpT[64 * h:64 * (h + 1), :]
        in_maps.append({"xT": xT, "wq": wq,
                        "kT": np.ascontiguousarray(kT),
                        "va": np.ascontiguousarray(va),
                        "wp": np.ascontiguousarray(wp)})
    return in_maps


def kernel(x, k, v, w_qkv, w_proj, b_proj):
    b_proj = np.asarray(b_proj, dtype=np.float32)

    if "nc" not in _cached:
        _cached["nc"] = build_program()
    nc = _cached["nc"]

    in_maps = build_in_maps(x, k, v, w_qkv, w_proj)
    res = run_bass_kernel_spmd(nc, in_maps, core_ids=list(range(NCORES)))

    out = np.empty((B, N, DIM), dtype=np.float32)
    for b in range(B):
        acc = np.zeros((N, DIM), dtype=np.float64)
        for core in range(4 * b, 4 * b + 4):
            acc += res.results[core]["out"]
        out[b] = (acc + b_proj).astype(np.float32)
    return out


# revision 18
# speedup vs baseline: 1.7944x; 1.0004x over previous
"""Multi-head attention (GAttention) on 8 trn2 NeuronCores.

Reference computation (per batch b):
    q = x @ w_qkv.T            -> [N, 768], heads of 64
    attn = softmax(q k^T / 8)  -> per head [N, M]
    out_h = attn @ v           -> [N, 64]
    out = concat(out_h) @ w_proj.T + b_proj

Sharding: 24 (b, head) units over 8 cores -> each core gets one batch b and
3 heads. Each core computes its heads' attention plus its partial
projection sum [N, 768]; host adds the 4 partials per batch + bias.

Per-core device pipeline:
  1. qproj (f32r): qT_dup[128, N] per head = [wq_h | wq_h]^T x^T; the
     duplicated column block makes rows 64:128 a copy of rows 0:64, which
     feeds the row-packed S^T matmuls.
  2. attention (bf16 operands, f32 PSUM), 6 (head, n-half) units; per key
     m-tile PAIR (2 x 128 keys, PE row groups 0/64 run concurrently):
       S^T = k q^T   -> PSUM [128, 2, 512] per n-chunk (tile A/B)
       expT = exp(0.125 S^T) -> SBUF bf16 (ACT, fused scale)
       AV: av[128, 1024] += v_aug^T expT   (accumulate over all 16 m-tiles)
     v_aug = [v_h | ones*64] so av rows 64:128 hold the softmax denominator.
  3. normalize: outTn (both partition halves) = av[0:64] * recip(av[64:128])
  4. proj (f32r): row-packed n-tile pairs, PSUM accumulates the 3 heads.
"""
import numpy as np
import ml_dtypes
from contextlib import ExitStack

import concourse.bass as bass
import concourse.mybir as mybir
import concourse.tile as tile
from concourse import bacc
from concourse.bass_utils import run_bass_kernel_spmd

B, N, DIM = 2, 2048, 768
H, D = 12, 64
M = 2048
NCORES = 8
HPC = 3            # heads per core
NT = N // 128      # 16 query tiles
MT = M // 128      # 16 key tiles
MP = MT // 2       # 8 key-tile pairs
CT = DIM // 128    # 6 contraction tiles for qproj
NHALF = 1024       # AV psum n-granularity
F32 = mybir.dt.float32
F32R = mybir.dt.float32r
BF16 = mybir.dt.bfloat16

_cached = {}

# dtype config: "fast" = bf16 attention+qproj, "mid" = f32r qproj + bf16 attn,
# "safe" = all f32r
import os
QUALITY = os.environ.get("KQ", "fast")
QP_DT = BF16 if QUALITY == "fast" else F32R
AT_DT = F32R if QUALITY == "safe" else BF16


def build_program():
    nc = bacc.Bacc("TRN2", target_bir_lowering=False, debug=False)
    xT_d = nc.dram_tensor("xT", [DIM, N], QP_DT, kind="ExternalInput")
    wq_d = nc.dram_tensor("wq", [HPC, DIM, 128], QP_DT, kind="ExternalInput")
    kT_d = nc.dram_tensor("kT", [128, HPC, MP, 128], AT_DT,
                          kind="ExternalInput")
    va_d = nc.dram_tensor("va", [HPC, M, 128], AT_DT, kind="ExternalInput")
    wp_d = nc.dram_tensor("wp", [128, HPC, DIM], F32R, kind="ExternalInput")
    out_d = nc.dram_tensor("out", [N, DIM], F32, kind="ExternalOutput")

    with tile.TileContext(nc) as tc, ExitStack() as ctx:
        big = ctx.enter_context(tc.tile_pool(name="big", bufs=1))
        expp = ctx.enter_context(tc.tile_pool(name="expp", bufs=4))
        stg = ctx.enter_context(tc.tile_pool(name="stg", bufs=3))

        # persistent SBUF tensors
        xT_t = [big.tile([128, N], QP_DT, name=f"xT{c}", tag=f"xT{c}")
                for c in range(CT)]
        for c in range(CT):
            nc.sync.dma_start(xT_t[c][:], xT_d[c * 128:(c + 1) * 128, :])
        wq_t = big.tile([128, HPC, CT, 128], QP_DT)
        nc.sync.dma_start(
            wq_t[:], wq_d.rearrange("h (c p) d -> p h c d", p=128))
        kT_t = big.tile([128, HPC, MP, 128], AT_DT)
        nc.sync.dma_start(kT_t[:], kT_d[:])
        va_t = big.tile([128, HPC, MT, 128], AT_DT)
        nc.sync.dma_start(
            va_t[:], va_d.rearrange("h (t p) e -> p h t e", p=128))
        wp_t = big.tile([128, HPC, DIM], F32R)
        nc.sync.dma_start(wp_t[:], wp_d[:])
        qT_t = big.tile([128, HPC, N], AT_DT)
        outTn_t = big.tile([128, HPC, N], F32R)

        # phase 1: q projection; wq has the head slice duplicated so rows
        # 64:128 of qT_t replicate rows 0:64
        with tc.tile_pool(name="qp_ps", bufs=1, space="PSUM") as qp_ps:
            for h in range(HPC):
                qp = qp_ps.tile([128, N], F32)
                for c in range(CT):
                    for ch in range(N // 512):
                        nc.tensor.matmul(
                            qp[:, ch * 512:(ch + 1) * 512],
                            wq_t[:, h, c, :],
                            xT_t[c][:, ch * 512:(ch + 1) * 512],
                            start=(c == 0), stop=(c == CT - 1),
                        )
                nc.vector.tensor_copy(qT_t[:, h, :], qp[:])

        # phase 2: attention in 6 (head, n-half) units; m-tile pairs are
        # row-packed on the PE (row groups 0 and 64). The AV matmuls for
        # iteration i are issued AFTER iteration i+1's S^T so the in-order
        # PE queue never stalls behind the EXP wait.
        with tc.tile_pool(name="st_ps", bufs=2, space="PSUM") as st_ps, \
             tc.tile_pool(name="av_ps", bufs=2, space="PSUM") as av_ps:
            av_by_unit = {}

            def _av(pend):
                unit, et, p, cc, first, last = pend[:6]
                av = av_by_unit[unit]
                nc.tensor.matmul(
                    av[:, cc * 512:(cc + 1) * 512],
                    va_t[:, unit[0], 2 * p, :], et[:, 0, :],
                    start=first, stop=False,
                )
                nc.tensor.matmul(
                    av[:, cc * 512:(cc + 1) * 512],
                    va_t[:, unit[0], 2 * p + 1, :], et[:, 1, :],
                    start=False, stop=last,
                )

            def _norm(unit):
                h, half = unit
                av = av_by_unit[unit]
                dn = expp.tile([64, NHALF], F32, tag="dn", name="dn")
                nc.vector.tensor_copy(dn[:], av[64:128, :])
                rs = expp.tile([64, NHALF], F32, tag="rs", name="rs")
                nc.vector.reciprocal_approx_fast(rs[:], dn[:])
                nsl = slice(half * NHALF, (half + 1) * NHALF)
                nc.vector.tensor_mul(
                    outTn_t[0:64, h, nsl], av[0:64, :], rs[:])
                nc.vector.tensor_mul(
                    outTn_t[64:128, h, nsl], av[0:64, :], rs[:])

            iters = [(h, half, p, cc)
                     for h in range(HPC) for half in range(N // NHALF)
                     for p in range(MP) for cc in range(NHALF // 512)]
            pend = []
            LAG = 2

            def _flush(limit):
                while len(pend) > limit:
                    pd = pend.pop(0)
                    _av(pd)
                    if pd[6]:
                        _norm(pd[0])

            for h, half, p, cc in iters:
                unit = (h, half)
                if unit not in av_by_unit:
                    av_by_unit[unit] = av_ps.tile(
                        [128, NHALF], F32, tag="av", name="av")
                n0 = half * NHALF + cc * 512
                st = st_ps.tile([128, 2, 512], F32, tag="st", name="st")
                nc.tensor.matmul(
                    st[:, 0, :], kT_t[0:64, h, p, :],
                    qT_t[0:64, h, n0:n0 + 512],
                    start=True, stop=True, tile_position=(0, 0),
                )
                nc.tensor.matmul(
                    st[:, 1, :], kT_t[64:128, h, p, :],
                    qT_t[64:128, h, n0:n0 + 512],
                    start=True, stop=True, tile_position=(64, 0),
                )
                _flush(LAG - 1)
                et = expp.tile([128, 2, 512], AT_DT, tag="et", name="et")
                nc.scalar.activation(
                    et[:], st[:], mybir.ActivationFunctionType.Exp,
                    scale=float(D) ** -0.5,
                )
                pend.append((unit, et, p, cc, p == 0, p == MP - 1,
                             p == MP - 1 and cc == NHALF // 512 - 1))
            _flush(0)

        # phase 3: projection, row-packed n-tile pairs, PSUM accumulates
        # the 3 heads
        with tc.tile_pool(name="pj_ps", bufs=2, space="PSUM") as pj_ps:
            for nj in range(NT // 2):
                ppa = pj_ps.tile([128, 2, 512], F32, tag="ppa")
                ppb = pj_ps.tile([128, 2, 512], F32, tag="ppb")
                na = 2 * nj * 128
                nb = (2 * nj + 1) * 128
                for h in range(HPC):
                    for oc in range(2):
                        osl = slice(oc * 384, (oc + 1) * 384)
                        nc.tensor.matmul(
                            ppa[:, oc, 0:384],
                            outTn_t[0:64, h, na:na + 128],
                            wp_t[0:64, h, osl],
                            start=(h == 0), stop=(h == HPC - 1),
                            tile_position=(0, 0),
                        )
                        nc.tensor.matmul(
                            ppb[:, oc, 0:384],
                            outTn_t[64:128, h, nb:nb + 128],
                            wp_t[64:128, h, osl],
                            start=(h == 0), stop=(h == HPC - 1),
                            tile_position=(64, 0),
                        )
                for which, pp, nn in ((0, ppa, na), (1, ppb, nb)):
                    os_t = stg.tile([128, DIM], F32, tag="os", name="os")
                    nc.vector.tensor_copy(os_t[:, 0:384], pp[:, 0, 0:384])
                    nc.vector.tensor_copy(os_t[:, 384:768], pp[:, 1, 0:384])
                    nc.sync.dma_start(out_d[nn:nn + 128, :], os_t[:])

    nc.compile()
    return nc


def build_in_maps(x, k, v, w_qkv, w_proj):
    x = np.asarray(x, dtype=np.float32)
    k = np.asarray(k, dtype=np.float32)
    v = np.asarray(v, dtype=np.float32)
    wqT = np.ascontiguousarray(np.asarray(w_qkv, np.float32).T)   # [C, 768]
    wpT = np.ascontiguousarray(np.asarray(w_proj, np.float32).T)  # [768, 768]

    in_maps = []
    for core in range(NCORES):
        b = core // 4
        hs = [3 * (core % 4) + i for i in range(HPC)]
        qp_np = ml_dtypes.bfloat16 if QUALITY == "fast" else np.float32
        at_np = np.float32 if QUALITY == "safe" else ml_dtypes.bfloat16
        xT = np.ascontiguousarray(x[b].T.astype(qp_np))
        # duplicated head slice -> qT rows 64:128 == rows 0:64
        wq = np.stack([
            np.concatenate([wqT[:, 64 * h:64 * (h + 1)]] * 2, axis=1)
            for h in hs]).astype(qp_np)                          # [3, DIM, 128]
        # kT layout [128, HPC, MP, 128]: rows 0:64 = head-dim of even m-tile,
        # rows 64:128 = head-dim of odd m-tile of each pair
        kb = k[b, hs].astype(at_np)                              # [3, M, D]
        kT = np.empty((128, HPC, MP, 128), dtype=at_np)
        for hi in range(HPC):
            for p in range(MP):
                kT[0:64, hi, p, :] = kb[hi, 256 * p:256 * p + 128, :].T
                kT[64:128, hi, p, :] = kb[hi, 256 * p + 128:256 * p + 256, :].T
        va = np.ones((HPC, M, 128), dtype=at_np)
        va[:, :, :D] = v[b, hs].astype(at_np)       # [3, M, 128]
        # wp duplicated on both partition halves for row-packed proj
        wp = np.empty((128, HPC, DIM), dtype=np.float32)
        for hi, h in enumerate(hs):
            wp[0:64, hi, :] = wpT[64 * h:64 * (h + 1), :]
            wp[64:128, hi, :] = wpT[64 * h:64 * (h + 1), :]
        in_maps.append({"xT": xT, "wq": wq,
                        "kT": np.ascontiguousarray(kT),
                        "va": np.ascontiguousarray(va),
                        "wp": np.ascontiguousarray(wp)})
    return in_maps


def kernel(x, k, v, w_qkv, w_proj, b_proj):
    b_proj = np.asarray(b_proj, dtype=np.float32)

    if "nc" not in _cached:
        _cached["nc"] = build_program()
    nc = _cached["nc"]

    in_maps = build_in_maps(x, k, v, w_qkv, w_proj)
    res = run_bass_kernel_spmd(nc, in_maps, core_ids=list(range(NCORES)))

    out = np.empty((B, N, DIM), dtype=np.float32)
    for b in range(B):
        acc = np.zeros((N, DIM), dtype=np.float64)
        for core in range(4 * b, 4 * b + 4):
            acc += res.results[core]["out"]
        out[b] = (acc + b_proj).astype(np.float32)
    return out
